# revision 68
# baseline (speedup 1.0000x reference)
"""Trainium2 Bass kernel for causal self-attention with RoPE.

Problem shapes (hardcoded): B=2, L=2048, D=1024, N=16 heads, H=64.

Sharding (8 cores, fully collective-free): data-parallel over batch
(2 groups of 4 cores), tensor-parallel over heads within a group
(4 heads/core).  Each core:
  1. computes q,k for its 4 heads in h-major layout (transposed matmul
     orientation: lhsT = w columns, rhs = x^T), applies RoPE on-chip,
  2. computes v in L-major layout (normal orientation),
  3. runs causal flash-style attention with scores transposed
     (S^T[key, query]) so softmax sums ride a fused ones-column through
     the PV matmul (no transposes anywhere),
  4. computes its PARTIAL output projection: its 4 heads' attention
     outputs (256 contraction dims) times the matching 256-row slice of
     w_proj, giving a full [L, D] partial in bf16.
Host code reformats/shards inputs (transpose, bf16 cast, column
permutation, table replication) and sums the 4 partials per batch.
There is no cross-core communication anywhere, so each core's
execution span is independent of peer launch skew.

v2 (this revision): the whole program is software-pipelined so the
Act engine (exp is Act-only, 0.83 ns/col + 185 ns/call) is fed from
t=0 while the PE runs projection work in the gaps:
  - projection / RoPE / out-projection work is cut into small chunks
    (<= ~1 us of PE or DVE time each) and emitted between attention
    score-groups, so the PE instruction stream never blocks on the
    Act->PV dependency for long;
  - within a head, scores(g+1) is emitted BEFORE PV(g), hiding the exp
    latency behind the next score matmuls (PSUM scores pool bufs=2);
  - exp calls are merged across the 2 key blocks of a group whenever
    the garbage columns cost less than the 185 ns per-call overhead
    (A-type diagonal groups merge, B-type split);
  - engine placement by measured cost-model rates AND hardware
    legality (Pool cannot touch PSUM; DVE tensor-tensor reads at most
    one PSUM input): RoPE muls/shuffle/add on DVE (bf16 2x), v copy as
    ONE strided Act copy per L-chunk, out-proj PSUM->SBUF staging on
    DVE (alternating DVE/Act on the last tile's drain tail), softmax
    1/z broadcast on Pool, every DMA on the SP/sync queue (HWDGE; Pool
    SWDGE descriptor generation would eat Pool engine time);
  - output projections are deferred two segments (op(0) in segment 2,
    op(1)+op(2) in segment 3) because the last segment is exp-bound
    with PE slack;
  - the first loads are split dc-granular and the pre-loop runs v/qk
    chains as interleaved half-chains in DMA arrival order;
  - the K projection runs in fp8-e4m3 DoubleRow (2x128 of D contracted
    per instruction at 0.5 cycles/column, halving its PE cost): host
    supplies x and wk*64 quantized to e4m3, the 1/64 scale-back rides
    the PSUM-drain copy, and each head uses its own base-0 [64, QT]
    psum tile (walrus rejects DoubleRow outputs at partition base 64).
    Measured end-to-end rel err 1.52e-2 (q stays bf16; quantizing both
    sides would exceed the 2e-2 budget).
"""

import numpy as np
import ml_dtypes

B, L, D, N_HEADS, H = 2, 2048, 1024, 16, 64
HPC = 4          # heads per core
GROUP = 4        # cores per batch group
NCORES = 8
QT = 512         # query tile width (matmul free dim)
KB = 128         # key block (psum partition dim)
N_QT = L // QT   # 4 query tiles
N_DC = D // 128  # 8 contraction chunks
N_LC = L // 128  # 16 L chunks for v / output rows
WPR = HPC * H    # w_proj rows per core (256)
BF16 = ml_dtypes.bfloat16

_prog_cache = {}


def _build_program():
    if "nc" in _prog_cache:
        return _prog_cache["nc"]

    import concourse.bass as bass
    import concourse.mybir as mybir
    import concourse.tile as tile
    from concourse import bacc
    from contextlib import ExitStack

    bf = mybir.dt.bfloat16
    f32 = mybir.dt.float32
    fp8 = mybir.dt.float8e4
    DR = mybir.MatmulPerfMode.DoubleRow

    nc = bacc.Bacc(num_devices=NCORES)

    # host pre-packs everything into the SBUF layout: [128, ...free dims]
    xt = nc.dram_tensor("xt", [128, N_QT * N_DC * QT], bf, kind="ExternalInput")
    wqk = nc.dram_tensor("wqk", [128, N_DC * 2 * 128], bf, kind="ExternalInput")
    x8t = nc.dram_tensor("x8t", [128, N_QT * 4 * 2 * QT], fp8, kind="ExternalInput")
    wk8t = nc.dram_tensor("wk8t", [128, 4 * 2 * 256], fp8, kind="ExternalInput")
    wv = nc.dram_tensor("wv", [128, N_DC * HPC * H], bf, kind="ExternalInput")
    wp = nc.dram_tensor("wp", [128, 2 * D], bf, kind="ExternalInput")
    ctab = nc.dram_tensor("ctab", [128, L], bf, kind="ExternalInput")
    stab = nc.dram_tensor("stab", [128, L], bf, kind="ExternalInput")
    tri = nc.dram_tensor("tri", [128, 128], bf, kind="ExternalInput")
    out = nc.dram_tensor("out", [L, D], bf, kind="ExternalOutput")

    Exp = mybir.ActivationFunctionType.Exp
    Copy = mybir.ActivationFunctionType.Copy
    SCALE = 1.0 / 8.0  # 1/sqrt(H)
    KWS = 64.0  # host-side fp8 scale on wk
    XOR1 = [i ^ 1 for i in range(32)]

    with tile.TileContext(nc) as tc, ExitStack() as ctx:
        singles = ctx.enter_context(tc.tile_pool(name="singles", bufs=1))
        work = ctx.enter_context(tc.tile_pool(name="work", bufs=6))
        epool = ctx.enter_context(tc.tile_pool(name="epool", bufs=4))
        dpool = ctx.enter_context(tc.tile_pool(name="dpool", bufs=6))
        opool = ctx.enter_context(tc.tile_pool(name="opool", bufs=4))
        ps_scores = ctx.enter_context(
            tc.tile_pool(name="ps_scores", bufs=2, space="PSUM")
        )
        ps_pv = ctx.enter_context(tc.tile_pool(name="ps_pv", bufs=2, space="PSUM"))
        ps_proj = ctx.enter_context(
            tc.tile_pool(name="ps_proj", bufs=2, space="PSUM")
        )

        # ---- SBUF-resident tensors ----
        xt_sb = singles.tile([128, N_QT, N_DC, QT], bf)
        wqk_sb = singles.tile([128, N_DC, 2, 128], bf)
        x8_sb = singles.tile([128, N_QT, 4, 2, QT], fp8)
        wk8_sb = singles.tile([128, 4, 2, 256], fp8)
        wv_sb = singles.tile([128, N_DC, HPC * H], bf)
        wp_sb = singles.tile([128, 2, D], bf)
        ctab_sb = singles.tile([128, L], bf)
        stab_sb = singles.tile([128, L], bf)
        tri_sb = singles.tile([128, 128], bf)
        qk_roped = singles.tile([128, 4, L], bf)
        v_sb = singles.tile([128, N_LC, HPC * (H + 1)], bf)
        attn_all = singles.tile([128, 2, L], bf)

        # ---- input loads, all on the SP/sync HWDGE queue, ordered by
        # first use: wv half + x quarters feed the first v chains, then
        # wqk + rope tables for the qk chains, then the later slabs ----
        SLAB = N_DC * QT
        QTR = SLAB // 4
        WVH = N_DC * HPC * H // 2
        # the first v chain (lc 0) consumes wv and xt dc-by-dc, so the
        # leading pieces are split small to get the PE started ~1.5us
        # earlier; later pieces are progressively larger
        nc.sync.dma_start(
            out=wv_sb[:, 0:2, :],
            in_=wv[:, 0 : WVH // 2].rearrange("p (dc m) -> p dc m", dc=2),
        )
        for dc in range(2):
            nc.sync.dma_start(
                out=xt_sb[:, 0, dc, :],
                in_=xt[:, QT * dc : QT * (dc + 1)],
            )
        nc.sync.dma_start(
            out=wv_sb[:, 2:4, :],
            in_=wv[:, WVH // 2 : WVH].rearrange("p (dc m) -> p dc m", dc=2),
        )
        for q in range(1, 4):
            nc.sync.dma_start(
                out=xt_sb[:, 0, 2 * q : 2 * (q + 1), :],
                in_=xt[:, QTR * q : QTR * (q + 1)].rearrange(
                    "p (dc c) -> p dc c", dc=2
                ),
            )
        nc.sync.dma_start(
            out=wv_sb[:, 4:8, :],
            in_=wv[:, WVH : 2 * WVH].rearrange("p (dc m) -> p dc m", dc=4),
        )
        for half in range(2):
            nc.sync.dma_start(
                out=wqk_sb[:, 4 * half : 4 * (half + 1), :, :],
                in_=wqk[
                    :, 4 * 2 * 128 * half : 4 * 2 * 128 * (half + 1)
                ].rearrange("p (dc qc m) -> p dc qc m", dc=4, qc=2),
            )
        nc.sync.dma_start(
            out=wk8_sb,
            in_=wk8t[:, :].rearrange("p (g j m) -> p g j m", g=4, j=2),
        )
        nc.sync.dma_start(
            out=x8_sb[:, 0, :, :, :],
            in_=x8t[:, 0 : 8 * QT].rearrange("p (g j c) -> p g j c", g=4, j=2),
        )
        nc.sync.dma_start(out=ctab_sb, in_=ctab[:, :])
        nc.sync.dma_start(out=stab_sb, in_=stab[:, :])
        nc.sync.dma_start(out=tri_sb, in_=tri[:, :])
        # slab 1 feeds segment 0's projection chunks (~14us in); wp is
        # only needed by the first output projection in segment 2, so it
        # loads after slab 1
        nc.sync.dma_start(
            out=xt_sb[:, 1, :, :],
            in_=xt[:, SLAB : 2 * SLAB].rearrange("p (dc c) -> p dc c", dc=N_DC),
        )
        nc.sync.dma_start(
            out=x8_sb[:, 1, :, :, :],
            in_=x8t[:, 8 * QT : 16 * QT].rearrange(
                "p (g j c) -> p g j c", g=4, j=2
            ),
        )
        nc.sync.dma_start(
            out=wp_sb[:, :, :],
            in_=wp[:, :].rearrange("p (j m) -> p j m", j=2),
        )
        for lt in range(2, N_QT):
            nc.sync.dma_start(
                out=xt_sb[:, lt, :, :],
                in_=xt[:, SLAB * lt : SLAB * (lt + 1)].rearrange(
                    "p (dc c) -> p dc c", dc=N_DC
                ),
            )
            nc.sync.dma_start(
                out=x8_sb[:, lt, :, :, :],
                in_=x8t[:, 8 * QT * lt : 8 * QT * (lt + 1)].rearrange(
                    "p (g j c) -> p g j c", g=4, j=2
                ),
            )

        # ones columns interleaved into v (softmax denominator rides PV)
        for h in range(HPC):
            nc.vector.memset(v_sb[:, :, (H + 1) * h + H], 1.0)

        # PE p-state warm-up: the tensor engine runs at half speed until
        # ~3us of continuous execution, and the ramp resets on idle.  The
        # first real chains sit behind the initial DMA wait, so a block
        # of dummy matmuls on memset data (available immediately) keeps
        # the PE busy through that wait and hands the real work a fully
        # ramped engine.
        warm = work.tile([128, QT], bf, tag="qkbf")
        nc.vector.memset(warm, 0.0)
        wps = ps_scores.tile([128, 2 * QT], f32, tag="scores")
        for _ in range(8):
            nc.tensor.matmul(
                wps[0:1, 0:QT], lhsT=warm[:, 0:1], rhs=warm,
                start=True, stop=True,
            )

        # ---- chunk generators: small closures the pipeline interleaves ----

        def v_chunk(lc, copy_on_act=True):
            # v projection for one 128-row L chunk, split into two 4-dc
            # half-chain chunks; ONE strided copy into the
            # (H+1)-interleaved v layout at the end
            st = {}

            def f2():
                st["ps"] = ps_proj.tile([128, HPC * H], f32, tag="proj",
                                        name="psv")
                for dc in range(N_DC):
                    nc.tensor.matmul(
                        st["ps"],
                        lhsT=xt_sb[
                            :, lc // 4, dc, 128 * (lc % 4) : 128 * (lc % 4 + 1)
                        ],
                        rhs=wv_sb[:, dc, :],
                        start=(dc == 0),
                        stop=(dc == N_DC - 1),
                    )
                # copy on Act early (it has slack there; DVE is tight),
                # on DVE for the last tile's v (Act margin shrinks by then)
                dst = v_sb[:, lc, :].rearrange("p (h x) -> p h x", h=HPC)[
                    :, :, 0:H
                ]
                src = st["ps"].rearrange("p (h x) -> p h x", h=HPC)
                if copy_on_act:
                    nc.scalar.activation(out=dst, in_=src, func=Copy)
                else:
                    nc.vector.tensor_copy(dst, src)
            return [f2]

        def qk_chunks(lt, qc):
            # q/k projection chain + RoPE for one (lt, qc), split into 3
            # chunks so no single chunk puts >1us on PE or DVE
            lsl = slice(QT * lt, QT * (lt + 1))
            st = {}

            def a2():
                st["ps"] = ps_proj.tile([128, QT], f32, tag="proj",
                                        name="psqk")
                for dc in range(N_DC):
                    nc.tensor.matmul(
                        st["ps"],
                        lhsT=wqk_sb[:, dc, qc, :],
                        rhs=xt_sb[:, lt, dc, :],
                        start=(dc == 0),
                        stop=(dc == N_DC - 1),
                    )
                qk_bf = work.tile([128, QT], bf, tag="qkbf")
                nc.scalar.activation(out=qk_bf, in_=st["ps"], func=Copy)
                st["qk_bf"] = qk_bf

            def b():
                # rot[p] = qk_bf[p ^ 1] (adjacent even/odd partner swap)
                rot = work.tile([128, QT], bf, tag="rot")
                nc.vector.stream_shuffle(rot, st["qk_bf"], mask=XOR1)
                m1 = work.tile([128, QT], bf, tag="m1")
                nc.vector.tensor_mul(m1, st["qk_bf"], ctab_sb[:, lsl])
                st["rot"], st["m1"] = rot, m1

            def c():
                m2 = work.tile([128, QT], bf, tag="m2")
                nc.vector.tensor_mul(m2, st["rot"], stab_sb[:, lsl])
                nc.vector.tensor_add(qk_roped[:, qc, lsl], st["m1"], m2)

            return [a2, b, c]

        def k8_chunks(lt, kc):
            # k projection in fp8 DoubleRow: each instruction contracts
            # 2x128 of D at 0.5 cycles/column, halving the PE cost of the
            # k chains.  Host supplies x and wk*KWS quantized to e4m3.
            # DoubleRow refuses out.base_partition=64 (walrus ISA check),
            # so each head gets its own base-0 [64, QT] psum tile; head 0
            # drains via Act (with the 1/KWS scale-back), head 1 via a
            # DVE tensor_scalar mul (same scale-back, Act stays lighter).
            lsl = slice(QT * lt, QT * (lt + 1))
            st = {}

            def _chain(hh, pst):
                wsl = slice(128 * (kc - 2) + 64 * hh,
                            128 * (kc - 2) + 64 * hh + 64)
                for ch in range(2):
                    for g in range(4):
                        nc.tensor.matmul(
                            pst[:, 256 * ch : 256 * (ch + 1)],
                            lhsT=wk8_sb[:, g, :, wsl],
                            rhs=x8_sb[:, lt, g, :, 256 * ch : 256 * (ch + 1)],
                            start=(g == 0),
                            stop=(g == 3),
                            perf_mode=DR,
                        )

            def aA():
                ps = ps_proj.tile([64, QT], f32, tag="proj", name="psk8")
                _chain(0, ps)
                qk_bf = work.tile([128, QT], bf, tag="qkbf")
                nc.scalar.activation(
                    out=qk_bf[0:64, :], in_=ps, func=Copy, scale=1.0 / KWS
                )
                st["qk_bf"] = qk_bf

            def a():
                ps = ps_proj.tile([64, QT], f32, tag="proj", name="psk8")
                _chain(1, ps)
                nc.vector.tensor_scalar_mul(
                    st["qk_bf"][64:128, :], ps, 1.0 / KWS
                )

            def b():
                rot = work.tile([128, QT], bf, tag="rot")
                nc.vector.stream_shuffle(rot, st["qk_bf"], mask=XOR1)
                m1 = work.tile([128, QT], bf, tag="m1")
                nc.vector.tensor_mul(m1, st["qk_bf"], ctab_sb[:, lsl])
                st["rot"], st["m1"] = rot, m1

            def c():
                m2 = work.tile([128, QT], bf, tag="m2")
                nc.vector.tensor_mul(m2, st["rot"], stab_sb[:, lsl])
                nc.vector.tensor_add(qk_roped[:, kc, lsl], st["m1"], m2)

            return [aA, a, b, c]

        def op_chunks(tp):
            # partial output projection for tile tp (4 L-chunks x 2 column
            # halves); PSUM->SBUF staging on Pool, DMA per L-chunk on SP.
            # The last tile DMAs per half and rotates the staging copies
            # across DVE/Act/Pool so the drain tail is short.
            last = tp == N_QT - 1
            osb_ref = {}
            chunks = []
            for lc in range(4 * tp, 4 * (tp + 1)):
                lsl = slice(128 * lc, 128 * (lc + 1))
                for oc in range(2):
                    osl = slice(QT * oc, QT * (oc + 1))

                    def f(lc=lc, oc=oc, lsl=lsl, osl=osl):
                        if oc == 0:
                            osb_ref[lc] = opool.tile(
                                [128, D], bf, tag="osb", name="osb"
                            )
                        osb = osb_ref[lc]
                        ps = ps_proj.tile([128, QT], f32, tag="proj", name="pso")
                        for j in range(2):
                            nc.tensor.matmul(
                                ps,
                                lhsT=attn_all[:, j, lsl],
                                rhs=wp_sb[:, j, osl],
                                start=(j == 0),
                                stop=(j == 1),
                            )
                        # staging copy PSUM->SBUF: only DVE and Act can
                        # read PSUM (GPSIMD/Pool cannot, BIR verifier).
                        # The last tile alternates halves between DVE and
                        # Act so the drain tail runs on both in parallel.
                        if last and (2 * lc + oc) % 2 == 1:
                            nc.scalar.activation(
                                out=osb[:, osl], in_=ps, func=Copy
                            )
                        else:
                            nc.vector.tensor_copy(osb[:, osl], ps)
                        if last:
                            nc.sync.dma_start(out=out[lsl, osl], in_=osb[:, osl])
                        elif oc == 1:
                            nc.sync.dma_start(out=out[lsl, :], in_=osb)

                    chunks.append(f)
            return chunks

        # ---- attention head emission with PV lagging one group behind
        # scores, chunks drained between groups ----

        def attn_head(t, h, drain, pre=None):
            qc = h // 2
            kc = 2 + h // 2
            base = 64 * (h % 2)
            q_all = qk_roped[base : base + 64, qc, :]
            k_all = qk_roped[base : base + 64, kc, :]
            qsl = slice(QT * t, QT * (t + 1))
            po = ps_pv.tile([H + 1, QT], f32, tag="pv")
            n_kb = 4 * (t + 1)
            n_g = n_kb // 2
            prev = None

            def emit_pv(et, g):
                # j=1's exps sit left-aligned at column QT (see below)
                for j in range(2):
                    kb = 2 * g + j
                    lo = max(128 * kb - QT * t, 0)
                    rhs = et[:, lo:QT] if j == 0 else et[:, QT : 2 * QT - lo]
                    nc.tensor.matmul(
                        po[:, lo:QT],
                        lhsT=v_sb[:, kb, (H + 1) * h : (H + 1) * (h + 1)],
                        rhs=rhs,
                        start=(kb == 0),
                        stop=(kb == n_kb - 1),
                    )

            for g in range(n_g):
                pss = ps_scores.tile([128, 2 * QT], f32, tag="scores")
                et = epool.tile([128, 2 * QT], bf, tag="etile")
                lo0 = max(128 * (2 * g) - QT * t, 0)
                lo1 = max(128 * (2 * g + 1) - QT * t, 0)
                # scores: j=0 in place at [lo0:QT]; j=1 LEFT-ALIGNED at
                # column QT so the two valid regions are contiguous and
                # one exp call covers exactly the causal columns with no
                # garbage and no second 185ns call
                nc.tensor.matmul(
                    pss[:, lo0:QT],
                    lhsT=k_all[:, 128 * (2 * g) : 128 * (2 * g + 1)],
                    rhs=q_all[:, QT * t + lo0 : QT * (t + 1)],
                    start=True,
                    stop=True,
                )
                nc.tensor.matmul(
                    pss[:, QT : 2 * QT - lo1],
                    lhsT=k_all[:, 128 * (2 * g + 1) : 128 * (2 * g + 2)],
                    rhs=q_all[:, QT * t + lo1 : QT * (t + 1)],
                    start=True,
                    stop=True,
                )
                nc.scalar.activation(
                    out=et[:, lo0 : 2 * QT - lo1],
                    in_=pss[:, lo0 : 2 * QT - lo1],
                    func=Exp,
                    scale=SCALE,
                )
                for j, lo in ((0, lo0), (1, lo1)):
                    kb = 2 * g + j
                    if 128 * kb - QT * t >= -127:
                        # boundary block: zero strictly-masked entries;
                        # the diag square of j=1 starts at column QT
                        dsl = slice(lo0 if j == 0 else QT,
                                    (lo0 if j == 0 else QT) + 128)
                        nc.vector.tensor_mul(
                            et[:, dsl], et[:, dsl], tri_sb
                        )
                if prev is not None:
                    emit_pv(*prev)
                prev = (et, g)
                if g == 0 and pre is not None:
                    # previous head's deferred normalize: emitted here so
                    # the chunk staging copies already queued on DVE run
                    # before it (the ps_proj ring never waits on it)
                    pre()
                drain()
            emit_pv(*prev)

            # normalize: attn = po[0:64] * (1 / po[64]) via DVE recip +
            # Pool partition broadcast + DVE mul.  (A PE outer-product
            # broadcast is illegal here: the mul would read two PSUM
            # inputs, and DVE allows only one.)  Returned as a closure:
            # the caller defers it into the next head's first group.
            def _norm():
                if (t, h) == (N_QT - 1, HPC - 1):
                    # tail-latency-critical: pipeline the chain in column
                    # halves so op(3) rows unblock after the first half
                    hw = QT // 2
                    for c in range(2):
                        csl = slice(hw * c, hw * (c + 1))
                        qs2 = slice(QT * t + hw * c, QT * t + hw * (c + 1))
                        z0 = dpool.tile([1, hw], f32, tag="z0")
                        nc.vector.reciprocal(out=z0, in_=po[H : H + 1, csl])
                        rb = dpool.tile([H, hw], f32, tag="rb")
                        nc.gpsimd.partition_broadcast(rb, z0)
                        nc.vector.tensor_mul(
                            attn_all[base : base + H, h // 2, qs2],
                            po[0:H, csl],
                            rb,
                        )
                else:
                    z0 = dpool.tile([1, QT], f32, tag="z0")
                    nc.vector.reciprocal(out=z0, in_=po[H : H + 1, :])
                    rb = dpool.tile([H, QT], f32, tag="rb")
                    nc.gpsimd.partition_broadcast(rb, z0)
                    nc.vector.tensor_mul(
                        attn_all[base : base + H, h // 2, qsl], po[0:H, :], rb
                    )
            return _norm

        # ---- pre-loop: v + qk for tile 0.  Chains run as interleaved
        # half-chains (dc 0-3 of a pair of chunks, then dc 4-7) so the PE
        # consumes input pieces in DMA arrival order instead of stalling
        # mid-chain on the not-yet-landed second half of x / weights ----
        def v_half(ps, lc, half):
            for dc in range(4 * half, 4 * half + 4):
                nc.tensor.matmul(
                    ps,
                    lhsT=xt_sb[
                        :, lc // 4, dc, 128 * (lc % 4) : 128 * (lc % 4 + 1)
                    ],
                    rhs=wv_sb[:, dc, :],
                    start=(dc == 0),
                    stop=(dc == N_DC - 1),
                )

        def qk_half(ps, qc, half):
            for dc in range(4 * half, 4 * half + 4):
                nc.tensor.matmul(
                    ps,
                    lhsT=wqk_sb[:, dc, qc, :],
                    rhs=xt_sb[:, 0, dc, :],
                    start=(dc == 0),
                    stop=(dc == N_DC - 1),
                )

        for pair in range(2):
            psa = ps_proj.tile([128, HPC * H], f32, tag="proj", name="psv")
            psb = ps_proj.tile([128, HPC * H], f32, tag="proj", name="psv")
            for half in range(2):
                v_half(psa, 2 * pair, half)
                v_half(psb, 2 * pair + 1, half)
            for lc, ps in ((2 * pair, psa), (2 * pair + 1, psb)):
                nc.scalar.activation(
                    out=v_sb[:, lc, :].rearrange("p (h x) -> p h x", h=HPC)[
                        :, :, 0:H
                    ],
                    in_=ps.rearrange("p (h x) -> p h x", h=HPC),
                    func=Copy,
                )
        # q chains (bf16, interleaved half-chains over both q chunks),
        # then per head-pair the k chain in fp8 DoubleRow + its rope, so
        # heads 0-1's q (qc0) AND k (kc2) ropes finish first
        psa = ps_proj.tile([128, QT], f32, tag="proj", name="psqk")
        psb = ps_proj.tile([128, QT], f32, tag="proj", name="psqk")
        for half in range(2):
            qk_half(psa, 0, half)
            qk_half(psb, 1, half)
        for qc, ps in ((0, psa), (1, psb)):
            lsl = slice(0, QT)
            qk_bf = work.tile([128, QT], bf, tag="qkbf")
            nc.scalar.activation(out=qk_bf, in_=ps, func=Copy)
            rot = work.tile([128, QT], bf, tag="rot")
            nc.vector.stream_shuffle(rot, qk_bf, mask=XOR1)
            m1 = work.tile([128, QT], bf, tag="m1")
            nc.vector.tensor_mul(m1, qk_bf, ctab_sb[:, lsl])
            m2 = work.tile([128, QT], bf, tag="m2")
            nc.vector.tensor_mul(m2, rot, stab_sb[:, lsl])
            nc.vector.tensor_add(qk_roped[:, qc, lsl], m1, m2)
            for f in k8_chunks(0, qc + 2):
                f()

        # ---- main software pipeline over query tiles ----
        fin = None
        for t in range(N_QT):
            pending = []
            if t + 1 < N_QT:
                # next tile's qk chains and v chunks, interleaved; qc
                # order (0, 2, 1, 3) completes heads 0-1's q AND k first
                # so the next segment's first scores never wait on RoPE
                for i, qc in enumerate((0, 2, 1, 3)):
                    gen = qk_chunks if qc < 2 else k8_chunks
                    pending.extend(gen(t + 1, qc))
                    pending.extend(v_chunk(4 * (t + 1) + i))
            # output projections are deferred one extra segment: the last
            # segment is Act(exp)-bound with PE slack, so pushing op work
            # there keeps the earlier PE-bound segments shorter
            if t == 2:
                pending.extend(op_chunks(0))
            elif t == 3:
                pending.extend(op_chunks(1))
                pending.extend(op_chunks(2))

            n_groups = 4 * 2 * (t + 1)
            emitted = 0
            total = len(pending)
            gi = 0

            def drain():
                nonlocal emitted, gi
                gi += 1
                want = (total * gi + n_groups - 1) // n_groups
                while emitted < min(want, total):
                    pending[emitted]()
                    emitted += 1

            for h in range(HPC):
                fin = attn_head(t, h, drain, pre=fin)
            while emitted < total:
                pending[emitted]()
                emitted += 1

        # ---- tail: output projection for the last tile ----
        fin()
        for f in op_chunks(N_QT - 1):
            f()

    nc.compile()
    _prog_cache["nc"] = nc
    return nc


def _host_inputs(x, rope, w_qkv, w_proj):
    """Shard + reformat the full inputs for the 8 cores."""
    rope = np.asarray(rope, dtype=np.float32)
    x = np.asarray(x, dtype=np.float32)
    w_qkv = np.asarray(w_qkv, dtype=np.float32)
    w_proj = np.asarray(w_proj, dtype=np.float32)

    # xt packed as [128, lt, dc, c]: xt[p, lt, dc, c] = x[b][lt*512+c, dc*128+p]
    xt_b = []
    for b in range(B):
        xb = x[b].T.reshape(N_DC, 128, N_QT, QT)  # [dc, p, lt, c]
        xt_b.append(
            np.ascontiguousarray(xb.transpose(1, 2, 0, 3))
            .reshape(128, N_QT * N_DC * QT)
            .astype(BF16)
        )

    # rope tables in h-major chunk layout: partition p of a 2-head chunk is
    # head (p // 64), component (p % 64); pair index i = (p % 64) // 2
    i_of_p = (np.arange(128) % 64) // 2
    cos_li = rope[:, :, 0]  # (L, 32)
    sin_li = rope[:, :, 1]
    ctab = np.ascontiguousarray(cos_li[:, i_of_p].T).astype(BF16)
    sign = np.where(np.arange(128) % 2 == 0, -1.0, 1.0).astype(np.float32)
    stab = np.ascontiguousarray((sin_li[:, i_of_p] * sign[None, :]).T).astype(BF16)

    # tri[p, f] = 1 where key offset p <= query offset f (keep), else 0
    tri = (np.arange(128)[:, None] <= np.arange(128)[None, :]).astype(BF16)

    # x in fp8 e4m3, packed for the DoubleRow k-projection rhs:
    # x8[p, lt, g, j, c] = x[b][512*lt + c, 128*(2g+j) + p]
    E4 = ml_dtypes.float8_e4m3
    x8_b = []
    for b in range(B):
        xq = x[b].reshape(N_QT, QT, 4, 2, 128)  # [lt, c, g, j, p]
        x8_b.append(
            np.ascontiguousarray(xq.transpose(4, 0, 2, 3, 1))
            .reshape(128, N_QT * 4 * 2 * QT)
            .astype(E4)
        )

    in_maps = []
    for c in range(NCORES):
        b, g = divmod(c, GROUP)
        heads = [HPC * g + i for i in range(HPC)]
        wq = np.concatenate([w_qkv[:, H * n : H * (n + 1)] for n in heads], 1)
        wk = np.concatenate(
            [w_qkv[:, D + H * n : D + H * (n + 1)] for n in heads], 1
        )
        wvv = np.concatenate(
            [w_qkv[:, 2 * D + H * n : 2 * D + H * (n + 1)] for n in heads], 1
        )
        # wqk packed as [128, dc, qc, m] (q only; k goes fp8), wv as
        # [128, dc, m]; wp as [128, j, m]
        wqk_p = (
            wq.reshape(N_DC, 128, 2, 128)
            .transpose(1, 0, 2, 3)
            .reshape(128, N_DC * 2 * 128)
        )
        # wk * KWS in fp8, DoubleRow lhsT layout:
        # wk8[p, g, j, m] = wk[128*(2g+j) + p, m] * KWS
        wk8_p = (
            (wk * 64.0)
            .reshape(4, 2, 128, WPR)
            .transpose(2, 0, 1, 3)
            .reshape(128, 4 * 2 * WPR)
            .astype(E4)
        )
        wv_p = (
            wvv.reshape(N_DC, 128, HPC * H)
            .transpose(1, 0, 2)
            .reshape(128, N_DC * HPC * H)
        )
        wp_p = (
            w_proj[WPR * g : WPR * (g + 1), :]
            .reshape(2, 128, D)
            .transpose(1, 0, 2)
            .reshape(128, 2 * D)
        )
        in_maps.append(
            {
                "xt": xt_b[b],
                "x8t": x8_b[b],
                "wk8t": np.ascontiguousarray(wk8_p),
                "wqk": np.ascontiguousarray(wqk_p).astype(BF16),
                "wv": np.ascontiguousarray(wv_p).astype(BF16),
                "wp": np.ascontiguousarray(wp_p).astype(BF16),
                "ctab": ctab,
                "stab": stab,
                "tri": tri,
            }
        )
    return in_maps


def kernel(x, rope, mask, w_qkv, w_proj, _trace=False):
    from concourse.bass_utils import run_bass_kernel_spmd

    nc = _build_program()
    in_maps = _host_inputs(x, rope, w_qkv, w_proj)
    res = run_bass_kernel_spmd(
        nc, in_maps, core_ids=list(range(NCORES)), trace=_trace
    )
    _prog_cache["last_result"] = res

    full = np.empty((B, L, D), dtype=np.float32)
    for b in range(B):
        acc = np.zeros((L, D), dtype=np.float32)
        for g in range(GROUP):
            acc += np.asarray(res.results[GROUP * b + g]["out"], dtype=np.float32)
        full[b] = acc
    return full
